# Initial kernel scaffold
#
"""Multi-head attention + residual + batchnorm on 8 trn2 NeuronCores.

Sharding: core c handles batch b = c % 4 and head-group g = c // 4
(4 heads = 512 feature dims per group). All device compute happens in
feature-major ("transposed") space so every matmul contracts over the
partition dim with zero on-chip transposes:

  QT[u,t] = (Wq_g/sqrt(D)) @ query[b].T      (fp32r)
  KT[u,t] = Wk_g @ keys[b].T                 (fp32r)
  V[t,u]  = keys[b] @ Wv_g.T                 (fp32r in, f32 psum, bf16 out)
  ST[k,q] = KT_h.T-contract QT_h             (fp32r; scores transposed)
  PT      = exp(ST)            (ACT, PSUM->SBUF, bf16; scores bounded, no max)
  OT[u,q] = sum_k V[k,u]*PT[k,q]             (bf16)
  r[q]    = sum_k PT[k,q]  via ones-matmul   (bf16)
  o_res   = OT/r + query[b].T slice          (f32)
  batchnorm over (b,s): local sums + 4-core AllReduce, then affine.

BatchNorm feature stats are local to a head-group, reduced across the 4
cores sharing g (replica groups [[0..3],[4..7]]).
"""
import sys

sys.path.insert(0, "/opt/trn_rl_repo")

import numpy as np

import concourse.bass as bass
import concourse.tile as tile
from concourse import bacc, mybir
from concourse.bass_utils import run_bass_kernel_spmd

F32 = mybir.dt.float32
F32R = mybir.dt.float32r
BF16 = mybir.dt.bfloat16
AF = mybir.ActivationFunctionType

B, S, D, H = 4, 2048, 1024, 8
DH = D // H          # 128
HG = 4               # heads per group (per core)
GF = HG * DH         # 512 features per group
EPS = 1e-5
P = 128
DT = 8               # d-tiles (D / 128)
TC = 4               # token chunks of 512
TCW = 512
KT_N = 16            # k tiles of 128 per sequence
NTOK = B * S         # batchnorm population per feature


def _build():
    nc = bacc.Bacc()
    qt = nc.declare_dram_parameter("qt", [D, S], F32, isOutput=False)
    kt = nc.declare_dram_parameter("kt", [D, S], F32, isOutput=False)
    wq = nc.declare_dram_parameter("wq", [D, GF], F32, isOutput=False)
    wk = nc.declare_dram_parameter("wk", [D, GF], F32, isOutput=False)
    wv = nc.declare_dram_parameter("wv", [D, GF], F32, isOutput=False)
    qres = nc.declare_dram_parameter("qres", [GF, S], F32, isOutput=False)
    gamma = nc.declare_dram_parameter("gamma", [P, HG], F32, isOutput=False)
    beta = nc.declare_dram_parameter("beta", [P, HG], F32, isOutput=False)
    out = nc.declare_dram_parameter("out", [P, HG, S], F32, isOutput=True)

    qt_t = qt.rearrange("(dt p) t -> p dt t", p=P)      # (128, 8, 2048)
    kt_t = kt.rearrange("(dt p) t -> p dt t", p=P)
    qres_t = qres.rearrange("(h p) t -> p h t", p=P)    # (128, 4, 2048)

    with tile.TileContext(nc) as tc:
        with (
            tc.tile_pool(name="persist", bufs=1) as persist,
            tc.tile_pool(name="dram", bufs=1, space="DRAM") as dram,
        ):
            # ---- persistent SBUF ----
            QT = persist.tile([P, HG, S], F32R)          # (dh, h, q) 32KB/p
            KTb = persist.tile([P, HG, S], F32R)         # (dh, h, k) 32KB/p
            V = persist.tile([P, KT_N, GF], BF16)        # (t128, kt, u) 16KB/p
            o_res = persist.tile([P, HG, S], F32)        # 32KB/p
            gam = persist.tile([P, HG], F32)
            bet = persist.tile([P, HG], F32)
            ones_b = persist.tile([P, 1], BF16)
            stats = persist.tile([P, 2 * HG], F32)       # [sum, sumsq] x heads
            cc_in = dram.tile([P, 2 * HG], F32)
            cc_out = dram.tile([P, 2 * HG], F32)

            nc.sync.dma_start(gam[:], gamma[:])
            nc.sync.dma_start(bet[:], beta[:])
            nc.vector.memset(ones_b[:], 1.0)

            # ---- phase 1: projections (stream qt/kt, weights resident) ----
            with (
                tc.tile_pool(name="wpool", bufs=1) as wpool,
                tc.tile_pool(name="xstream", bufs=4) as xstream,
                tc.tile_pool(name="ppsum", bufs=8, space="PSUM") as ppsum,
                tc.tile_pool(name="ptmp", bufs=4) as ptmp,
            ):
                wq_s = wpool.tile([P, DT, GF], F32R)
                wk_s = wpool.tile([P, DT, GF], F32R)
                wv_s = wpool.tile([P, DT, GF], F32R)
                nc.gpsimd.dma_start(wq_s[:], wq.rearrange("(dt p) u -> p dt u", p=P))
                nc.gpsimd.dma_start(wk_s[:], wk.rearrange("(dt p) u -> p dt u", p=P))
                nc.gpsimd.dma_start(wv_s[:], wv.rearrange("(dt p) u -> p dt u", p=P))

                # Q^T
                for tc_i in range(TC):
                    x_ch = xstream.tile([P, DT, TCW], F32R, tag="xq")
                    nc.gpsimd.dma_start(
                        x_ch[:], qt_t[:, :, bass.ts(tc_i, TCW)]
                    )
                    for h in range(HG):
                        ps = ppsum.tile([P, TCW], F32)
                        for d in range(DT):
                            nc.tensor.matmul(
                                ps[:],
                                wq_s[:, d, bass.ts(h, DH)],
                                x_ch[:, d, :],
                                start=(d == 0),
                                stop=(d == DT - 1),
                            )
                        nc.vector.tensor_copy(
                            QT[:, h, bass.ts(tc_i, TCW)], ps[:]
                        )
                # K^T and V
                for tc_i in range(TC):
                    x_ch = xstream.tile([P, DT, TCW], F32R, tag="xk")
                    nc.gpsimd.dma_start(
                        x_ch[:], kt_t[:, :, bass.ts(tc_i, TCW)]
                    )
                    for h in range(HG):
                        ps = ppsum.tile([P, TCW], F32)
                        for d in range(DT):
                            nc.tensor.matmul(
                                ps[:],
                                wk_s[:, d, bass.ts(h, DH)],
                                x_ch[:, d, :],
                                start=(d == 0),
                                stop=(d == DT - 1),
                            )
                        nc.vector.tensor_copy(
                            KTb[:, h, bass.ts(tc_i, TCW)], ps[:]
                        )
                    for sub in range(TCW // P):  # 4 t128 tiles in this chunk
                        kt_idx = tc_i * (TCW // P) + sub
                        psv = ppsum.tile([P, GF], F32)
                        for d in range(DT):
                            nc.tensor.matmul(
                                psv[:],
                                x_ch[:, d, bass.ts(sub, P)],
                                wv_s[:, d, :],
                                start=(d == 0),
                                stop=(d == DT - 1),
                            )
                        nc.vector.tensor_copy(V[:, kt_idx, :], psv[:])

            # ---- phase 2: attention + residual ----
            with (
                tc.tile_pool(name="pt_pool", bufs=2) as pt_pool,
                tc.tile_pool(name="qr_pool", bufs=3) as qr_pool,
                tc.tile_pool(name="spsum", bufs=3, space="PSUM") as spsum,
                tc.tile_pool(name="opsum", bufs=2, space="PSUM") as opsum,
                tc.tile_pool(name="rpsum", bufs=2, space="PSUM") as rpsum,
                tc.tile_pool(name="small", bufs=4) as small,
            ):
                for h in range(HG):
                    for q_i in range(TC):
                        PT = pt_pool.tile([P, KT_N, TCW], BF16, tag="pt")
                        for k in range(KT_N):
                            ps_s = spsum.tile([P, TCW], F32, tag="s")
                            nc.tensor.matmul(
                                ps_s[:],
                                KTb[:, h, bass.ts(k, P)],
                                QT[:, h, bass.ts(q_i, TCW)],
                                start=True,
                                stop=True,
                            )
                            nc.scalar.activation(
                                out=PT[:, k, :], in_=ps_s[:], func=AF.Exp
                            )
                        ps_o = opsum.tile([P, TCW], F32, tag="o")
                        for k in range(KT_N):
                            nc.tensor.matmul(
                                ps_o[:],
                                V[:, k, bass.ts(h, DH)],
                                PT[:, k, :],
                                start=(k == 0),
                                stop=(k == KT_N - 1),
                            )
                        ps_r = rpsum.tile([1, TCW], F32, tag="r")
                        for k in range(KT_N):
                            nc.tensor.matmul(
                                ps_r[:],
                                ones_b[:],
                                PT[:, k, :],
                                start=(k == 0),
                                stop=(k == KT_N - 1),
                            )
                        rinv = small.tile([1, TCW], F32, tag="rinv")
                        nc.vector.reciprocal(out=rinv[:], in_=ps_r[:])
                        rb = bass.AP(
                            tensor=rinv.tensor,
                            offset=rinv.offset,
                            ap=[[0, P]] + list(rinv.ap[1:]),
                        )
                        qres_ch = qr_pool.tile([P, TCW], F32, tag="qres")
                        nc.sync.dma_start(
                            qres_ch[:], qres_t[:, h, bass.ts(q_i, TCW)]
                        )
                        dst = o_res[:, h, bass.ts(q_i, TCW)]
                        nc.vector.tensor_tensor(
                            dst, ps_o[:], rb, mybir.AluOpType.mult
                        )
                        nc.vector.tensor_add(dst, dst, qres_ch[:])

                # ---- batchnorm stats (local), then 4-core all-reduce ----
                for h in range(HG):
                    bstat = small.tile(
                        [P, TC, nc.vector.BN_STATS_DIM], F32, tag="bstat"
                    )
                    for sg in range(TC):
                        nc.vector.bn_stats(
                            out=bstat[:, sg, :],
                            in_=o_res[:, h, bass.ts(sg, TCW)],
                        )
                    mv = small.tile([P, 2], F32, tag="mv")
                    nc.vector.bn_aggr(out=mv[:], in_=bstat[:])
                    # sum = mean*S ; sumsq = (var + mean^2)*S
                    nc.vector.tensor_scalar_mul(
                        stats[:, h : h + 1], mv[:, 0:1], float(S)
                    )
                    sq = small.tile([P, 1], F32, tag="sq")
                    nc.vector.tensor_mul(sq[:], mv[:, 0:1], mv[:, 0:1])
                    nc.vector.tensor_add(sq[:], sq[:], mv[:, 1:2])
                    nc.vector.tensor_scalar_mul(
                        stats[:, HG + h : HG + h + 1], sq[:], float(S)
                    )

                nc.gpsimd.dma_start(cc_in[:], stats[:])
                nc.gpsimd.collective_compute(
                    "AllReduce",
                    mybir.AluOpType.add,
                    ins=[cc_in.opt()],
                    outs=[cc_out.opt()],
                    replica_groups=[[0, 1, 2, 3], [4, 5, 6, 7]],
                )
                gstat = small.tile([P, 2 * HG], F32, tag="gstat")
                nc.gpsimd.dma_start(gstat[:], cc_out[:])

                # mean = sum/NTOK ; var = sumsq/NTOK - mean^2
                mean = small.tile([P, HG], F32, tag="mean")
                var = small.tile([P, HG], F32, tag="var")
                nc.vector.tensor_scalar_mul(mean[:], gstat[:, :HG], 1.0 / NTOK)
                nc.vector.tensor_scalar_mul(var[:], gstat[:, HG:], 1.0 / NTOK)
                msq = small.tile([P, HG], F32, tag="msq")
                nc.vector.tensor_mul(msq[:], mean[:], mean[:])
                nc.vector.tensor_sub(var[:], var[:], msq[:])
                # rstd = 1/sqrt(var + eps)
                std = small.tile([P, HG], F32, tag="std")
                nc.scalar.activation(
                    out=std[:], in_=var[:], func=AF.Sqrt, bias=float(EPS)
                )
                rstd = small.tile([P, HG], F32, tag="rstd")
                nc.vector.reciprocal(out=rstd[:], in_=std[:])
                scale = small.tile([P, HG], F32, tag="scale")
                shift = small.tile([P, HG], F32, tag="shift")
                nc.vector.tensor_mul(scale[:], gam[:], rstd[:])
                nc.vector.tensor_mul(shift[:], mean[:], scale[:])
                nc.vector.tensor_sub(shift[:], bet[:], shift[:])

                # ---- final affine + writeout ----
                for h in range(HG):
                    nc.vector.tensor_scalar(
                        o_res[:, h, :],
                        o_res[:, h, :],
                        scale[:, h : h + 1],
                        shift[:, h : h + 1],
                        mybir.AluOpType.mult,
                        mybir.AluOpType.add,
                    )
                    nc.sync.dma_start(out[:, h, :], o_res[:, h, :])

    nc.finalize()
    return nc


_NC = None


def _get_nc():
    global _NC
    if _NC is None:
        _NC = _build()
    return _NC


def _make_in_maps(query, keys, Wq, Wk, Wv, gamma, beta):
    query = np.asarray(query, dtype=np.float32)
    keys = np.asarray(keys, dtype=np.float32)
    Wq = np.asarray(Wq, dtype=np.float32)
    Wk = np.asarray(Wk, dtype=np.float32)
    Wv = np.asarray(Wv, dtype=np.float32)
    gamma = np.asarray(gamma, dtype=np.float32)
    beta = np.asarray(beta, dtype=np.float32)

    scale = 1.0 / np.sqrt(np.float32(D))
    in_maps = []
    for c in range(8):
        b, g = c % B, c // B
        rows = slice(GF * g, GF * (g + 1))
        qt = np.ascontiguousarray(query[b].T)              # (D, S)
        kt = np.ascontiguousarray(keys[b].T)
        in_maps.append(
            {
                "qt": qt,
                "kt": kt,
                "wq": np.ascontiguousarray(Wq[rows].T * scale),  # (D, GF)
                "wk": np.ascontiguousarray(Wk[rows].T),
                "wv": np.ascontiguousarray(Wv[rows].T),
                "qres": np.ascontiguousarray(qt[rows]),          # (GF, S)
                "gamma": np.ascontiguousarray(
                    gamma[rows].reshape(HG, P).T
                ),
                "beta": np.ascontiguousarray(beta[rows].reshape(HG, P).T),
            }
        )
    return in_maps


def _run(in_maps, trace=False, **kw):
    nc = _get_nc()
    return run_bass_kernel_spmd(
        nc, in_maps, core_ids=list(range(8)), trace=trace, **kw
    )


def kernel(query, keys, Wq, Wk, Wv, gamma, beta):
    in_maps = _make_in_maps(query, keys, Wq, Wk, Wv, gamma, beta)
    res = _run(in_maps)
    output = np.empty((B, S, D), dtype=np.float32)
    for c in range(8):
        b, g = c % B, c // B
        oc = res.results[c]["out"]                   # (128, 4, 2048)
        block = oc.transpose(2, 1, 0).reshape(S, GF)  # (S, GF): [t, h*128+p]
        output[b, :, GF * g : GF * (g + 1)] = block
    return output


# revision 12
# speedup vs baseline: 1.3386x; 1.3386x over previous
"""Multi-head attention + residual + batchnorm on 8 trn2 NeuronCores.

Sharding: core c handles batch b = c % 4 and head-group g = c // 4
(4 heads = 512 feature dims per group). All device compute happens in
feature-major ("transposed") space so every matmul contracts over the
partition dim with zero on-chip transposes:

  QT[u,t] = (Wq_g/sqrt(D)) @ query[b].T      (fp32r)
  KT[u,t] = Wk_g @ keys[b].T                 (fp32r)
  V[t,u]  = keys[b] @ Wv_g.T                 (fp32r in, f32 psum, bf16 out)
  ST[k,q] = KT_h.T-contract QT_h             (fp32r; scores transposed)
  PT      = exp(ST)            (ACT, PSUM->SBUF, bf16; scores bounded, no max)
  OT[u,q] = sum_k V[k,u]*PT[k,q]             (bf16)
  r[q]    = sum_k PT[k,q]  via ones-matmul   (bf16)
  o_res   = OT/r + query[b].T slice          (f32)
  batchnorm over (b,s): local sums + 4-core AllReduce, then affine.

BatchNorm feature stats are local to a head-group, reduced across the 4
cores sharing g (replica groups [[0..3],[4..7]]).
"""
import sys

sys.path.insert(0, "/opt/trn_rl_repo")

import numpy as np

import concourse.bass as bass
import concourse.tile as tile
from concourse import bacc, mybir
from concourse.bass_utils import run_bass_kernel_spmd

F32 = mybir.dt.float32
F32R = mybir.dt.float32r
BF16 = mybir.dt.bfloat16
AF = mybir.ActivationFunctionType

B, S, D, H = 4, 2048, 1024, 8
DH = D // H          # 128
HG = 4               # heads per group (per core)
GF = HG * DH         # 512 features per group
EPS = 1e-5
P = 128
DT = 8               # d-tiles (D / 128)
TC = 4               # token chunks of 512
TCW = 512
KT_N = 16            # k tiles of 128 per sequence
NTOK = B * S         # batchnorm population per feature


def _build():
    nc = bacc.Bacc(num_swdge_queues=8)
    qt = nc.declare_dram_parameter("qt", [D, S], F32R, isOutput=False)
    kt = nc.declare_dram_parameter("kt", [D, S], F32R, isOutput=False)
    wq = nc.declare_dram_parameter("wq", [D, GF], F32R, isOutput=False)
    wk = nc.declare_dram_parameter("wk", [D, GF], F32R, isOutput=False)
    wv = nc.declare_dram_parameter("wv", [D, GF], F32R, isOutput=False)
    qres = nc.declare_dram_parameter("qres", [GF, S], F32, isOutput=False)
    gamma = nc.declare_dram_parameter("gamma", [P, HG], F32, isOutput=False)
    beta = nc.declare_dram_parameter("beta", [P, HG], F32, isOutput=False)
    out = nc.declare_dram_parameter("out", [P, HG, S], F32, isOutput=True)

    qt_t = qt.rearrange("(dt p) t -> p dt t", p=P)      # (128, 8, 2048)
    kt_t = kt.rearrange("(dt p) t -> p dt t", p=P)
    qres_t = qres.rearrange("(h p) t -> p h t", p=P)    # (128, 4, 2048)

    with tile.TileContext(nc) as tc:
        with (
            tc.tile_pool(name="persist", bufs=1) as persist,
            tc.tile_pool(name="dram", bufs=1, space="DRAM") as dram,
        ):
            # ---- persistent SBUF ----
            QT = persist.tile([P, HG, S], BF16)          # (dh, h, q) 16KB/p
            KTb = persist.tile([P, HG, S], BF16)         # (dh, h, k) 16KB/p
            V = persist.tile([P, KT_N, GF], BF16)        # (t128, kt, u) 16KB/p
            o_res = persist.tile([P, HG, S], F32)        # 32KB/p
            gam = persist.tile([P, HG], F32)
            bet = persist.tile([P, HG], F32)
            ones_f = persist.tile([P, P], F32)
            ones_b = persist.tile([P, P], F32R)
            stats = persist.tile([P, 2 * HG], F32)       # [sum, sumsq] x heads
            cc_in = dram.tile([P, 2 * HG], F32)
            cc_out = dram.tile([P, 2 * HG], F32)

            nc.sync.dma_start(gam[:], gamma[:])
            nc.sync.dma_start(bet[:], beta[:])
            nc.vector.memset(ones_f[:], 1.0)
            nc.vector.tensor_copy(ones_b[:], ones_f[:])

            # ---- phase 1: projections (stream qt/kt, weights resident) ----
            with (
                tc.tile_pool(name="wpool", bufs=1) as wpool,
                tc.tile_pool(name="xstream", bufs=3) as xstream,
                tc.tile_pool(name="ppsum", bufs=4, space="PSUM") as ppsum,
                tc.tile_pool(name="ptmp", bufs=4) as ptmp,
            ):
                wq_s = [wpool.tile([P, GF], F32R, name=f"wq{d}") for d in range(DT)]
                wk_s = [wpool.tile([P, GF], F32R, name=f"wk{d}") for d in range(DT)]
                wv_s = [wpool.tile([P, GF], F32R, name=f"wv{d}") for d in range(DT)]
                wq_r = wq.rearrange("(dt p) u -> dt p u", p=P)
                wk_r = wk.rearrange("(dt p) u -> dt p u", p=P)
                wv_r = wv.rearrange("(dt p) u -> dt p u", p=P)
                for d in range(DT):
                    nc.sync.dma_start(wq_s[d][:], wq_r[d])

                # Q^T
                for tc_i in range(TC):
                    xh = []
                    if tc_i == 1:
                        for d in range(DT):
                            nc.sync.dma_start(wk_s[d][:], wk_r[d])
                    elif tc_i == 2:
                        for d in range(DT):
                            nc.sync.dma_start(wv_s[d][:], wv_r[d])
                    for half in range(2):
                        t = xstream.tile([P, DT // 2, TCW], F32R, tag="x")
                        nc.sync.dma_start(
                            t[:],
                            qt_t[:, bass.ts(half, DT // 2), bass.ts(tc_i, TCW)],
                        )
                        xh.append(t)
                    for h in range(HG):
                        ps = ppsum.tile([P, TCW], F32)
                        for d in range(DT):
                            nc.tensor.matmul(
                                ps[:],
                                wq_s[d][:, bass.ts(h, DH)],
                                xh[d // 4][:, d % 4, :],
                                start=(d == 0),
                                stop=(d == DT - 1),
                            )
                        nc.scalar.copy(
                            QT[:, h, bass.ts(tc_i, TCW)], ps[:]
                        )
                # K^T and V
                for tc_i in range(TC):
                    xh = []
                    for half in range(2):
                        t = xstream.tile([P, DT // 2, TCW], F32R, tag="x")
                        nc.sync.dma_start(
                            t[:],
                            kt_t[:, bass.ts(half, DT // 2), bass.ts(tc_i, TCW)],
                        )
                        xh.append(t)
                    for h in range(HG):
                        ps = ppsum.tile([P, TCW], F32)
                        for d in range(DT):
                            nc.tensor.matmul(
                                ps[:],
                                wk_s[d][:, bass.ts(h, DH)],
                                xh[d // 4][:, d % 4, :],
                                start=(d == 0),
                                stop=(d == DT - 1),
                            )
                        nc.scalar.copy(
                            KTb[:, h, bass.ts(tc_i, TCW)], ps[:]
                        )
                    for sub in range(TCW // P):  # 4 t128 tiles in this chunk
                        kt_idx = tc_i * (TCW // P) + sub
                        psv = ppsum.tile([P, GF], F32)
                        for d in range(DT):
                            nc.tensor.matmul(
                                psv[:],
                                xh[d // 4][:, d % 4, bass.ts(sub, P)],
                                wv_s[d][:],
                                start=(d == 0),
                                stop=(d == DT - 1),
                            )
                        nc.scalar.copy(V[:, kt_idx, :], psv[:])

            # ---- phase 2: attention + residual ----
            with (
                tc.tile_pool(name="pt_pool", bufs=2) as pt_pool,
                tc.tile_pool(name="qr_pool", bufs=3) as qr_pool,
                tc.tile_pool(name="spsum", bufs=2, space="PSUM") as spsum,
                tc.tile_pool(name="opsum", bufs=2, space="PSUM") as opsum,
                tc.tile_pool(name="rpsum", bufs=2, space="PSUM") as rpsum,
                tc.tile_pool(name="small", bufs=4) as small,
            ):
                for h in range(HG):
                    for q_i in range(TC):
                        PT = pt_pool.tile([P, KT_N, TCW], BF16, tag="pt")
                        for kp in range(KT_N // 2):
                            ps_s = spsum.tile([P, 2, TCW], F32, tag="s")
                            for j in range(2):
                                nc.tensor.matmul(
                                    ps_s[:, j, :],
                                    KTb[:, h, bass.ts(2 * kp + j, P)],
                                    QT[:, h, bass.ts(q_i, TCW)],
                                    start=True,
                                    stop=True,
                                )
                            nc.scalar.activation(
                                out=PT[:, 2 * kp : 2 * kp + 2, :],
                                in_=ps_s[:],
                                func=AF.Exp,
                            )
                        ps_o = opsum.tile([P, TCW], F32, tag="o")
                        for k in range(KT_N):
                            nc.tensor.matmul(
                                ps_o[:],
                                V[:, k, bass.ts(h, DH)],
                                PT[:, k, :],
                                start=(k == 0),
                                stop=(k == KT_N - 1),
                            )
                        # pairwise in-place tree over the 16 k-tiles of PT
                        for step in (1, 2, 4, 8):
                            for j in range(0, KT_N, 2 * step):
                                if step < 8:
                                    nc.vector.tensor_add(
                                        PT[:, j, :], PT[:, j, :], PT[:, j + step, :]
                                    )
                        rpart = small.tile([P, TCW], F32R, tag="rpart")
                        nc.vector.tensor_add(
                            rpart[:], PT[:, 0, :], PT[:, 8, :]
                        )
                        ps_r = rpsum.tile([P, TCW], F32, tag="r")
                        nc.tensor.matmul(
                            ps_r[:], ones_b[:], rpart[:], start=True, stop=True
                        )
                        rb = small.tile([P, TCW], F32, tag="rb")
                        nc.vector.reciprocal_approx_fast(out=rb[:], in_=ps_r[:])
                        qres_ch = qr_pool.tile([P, TCW], F32, tag="qres")
                        nc.sync.dma_start(
                            qres_ch[:], qres_t[:, h, bass.ts(q_i, TCW)]
                        )
                        dst = o_res[:, h, bass.ts(q_i, TCW)]
                        nc.vector.tensor_tensor(
                            dst, ps_o[:], rb[:], mybir.AluOpType.mult
                        )
                        nc.vector.tensor_add(dst, dst, qres_ch[:])

                    # batchnorm stats for this head (overlaps next head)
                    bstat = small.tile(
                        [P, TC, nc.vector.BN_STATS_DIM], F32, tag="bstat"
                    )
                    for sg in range(TC):
                        nc.vector.bn_stats(
                            out=bstat[:, sg, :],
                            in_=o_res[:, h, bass.ts(sg, TCW)],
                        )
                    mv = small.tile([P, 2], F32, tag="mv")
                    nc.vector.bn_aggr(out=mv[:], in_=bstat[:])
                    # sum = mean*S ; sumsq = (var + mean^2)*S
                    nc.vector.tensor_scalar_mul(
                        stats[:, h : h + 1], mv[:, 0:1], float(S)
                    )
                    sq = small.tile([P, 1], F32, tag="sq")
                    nc.vector.tensor_mul(sq[:], mv[:, 0:1], mv[:, 0:1])
                    nc.vector.tensor_add(sq[:], sq[:], mv[:, 1:2])
                    nc.vector.tensor_scalar_mul(
                        stats[:, HG + h : HG + h + 1], sq[:], float(S)
                    )

                nc.gpsimd.dma_start(cc_in[:], stats[:])
                nc.gpsimd.collective_compute(
                    "AllReduce",
                    mybir.AluOpType.add,
                    ins=[cc_in.opt()],
                    outs=[cc_out.opt()],
                    replica_groups=[[0, 1, 2, 3], [4, 5, 6, 7]],
                )
                gstat = small.tile([P, 2 * HG], F32, tag="gstat")
                nc.gpsimd.dma_start(gstat[:], cc_out[:])

                # mean = sum/NTOK ; var = sumsq/NTOK - mean^2
                mean = small.tile([P, HG], F32, tag="mean")
                var = small.tile([P, HG], F32, tag="var")
                nc.vector.tensor_scalar_mul(mean[:], gstat[:, :HG], 1.0 / NTOK)
                nc.vector.tensor_scalar_mul(var[:], gstat[:, HG:], 1.0 / NTOK)
                msq = small.tile([P, HG], F32, tag="msq")
                nc.vector.tensor_mul(msq[:], mean[:], mean[:])
                nc.vector.tensor_sub(var[:], var[:], msq[:])
                # rstd = 1/sqrt(var + eps)
                eps_t = small.tile([P, 1], F32, tag="eps")
                nc.vector.memset(eps_t[:], float(EPS))
                std = small.tile([P, HG], F32, tag="std")
                nc.scalar.activation(
                    out=std[:], in_=var[:], func=AF.Sqrt, bias=eps_t[:]
                )
                rstd = small.tile([P, HG], F32, tag="rstd")
                nc.vector.reciprocal(out=rstd[:], in_=std[:])
                scale = small.tile([P, HG], F32, tag="scale")
                shift = small.tile([P, HG], F32, tag="shift")
                nc.vector.tensor_mul(scale[:], gam[:], rstd[:])
                nc.vector.tensor_mul(shift[:], mean[:], scale[:])
                nc.vector.tensor_sub(shift[:], bet[:], shift[:])

                # ---- final affine + writeout ----
                for h in range(HG):
                    nc.vector.tensor_scalar(
                        o_res[:, h, :],
                        o_res[:, h, :],
                        scale[:, h : h + 1],
                        shift[:, h : h + 1],
                        mybir.AluOpType.mult,
                        mybir.AluOpType.add,
                    )
                    nc.sync.dma_start(out[:, h, :], o_res[:, h, :])

    nc.finalize()
    return nc


_NC = None


def _get_nc():
    global _NC
    if _NC is None:
        _NC = _build()
    return _NC


def _make_in_maps(query, keys, Wq, Wk, Wv, gamma, beta):
    query = np.asarray(query, dtype=np.float32)
    keys = np.asarray(keys, dtype=np.float32)
    Wq = np.asarray(Wq, dtype=np.float32)
    Wk = np.asarray(Wk, dtype=np.float32)
    Wv = np.asarray(Wv, dtype=np.float32)
    gamma = np.asarray(gamma, dtype=np.float32)
    beta = np.asarray(beta, dtype=np.float32)

    scale = 1.0 / np.sqrt(np.float32(D))
    in_maps = []
    for c in range(8):
        b, g = c % B, c // B
        rows = slice(GF * g, GF * (g + 1))
        qt = np.ascontiguousarray(query[b].T)              # (D, S)
        kt = np.ascontiguousarray(keys[b].T)
        in_maps.append(
            {
                "qt": qt,
                "kt": kt,
                "wq": np.ascontiguousarray(Wq[rows].T * scale),  # (D, GF)
                "wk": np.ascontiguousarray(Wk[rows].T),
                "wv": np.ascontiguousarray(Wv[rows].T),
                "qres": np.ascontiguousarray(qt[rows]),          # (GF, S)
                "gamma": np.ascontiguousarray(
                    gamma[rows].reshape(HG, P).T
                ),
                "beta": np.ascontiguousarray(beta[rows].reshape(HG, P).T),
            }
        )
    return in_maps


def _run(in_maps, trace=False, **kw):
    nc = _get_nc()
    return run_bass_kernel_spmd(
        nc, in_maps, core_ids=list(range(8)), trace=trace, **kw
    )


def kernel(query, keys, Wq, Wk, Wv, gamma, beta):
    in_maps = _make_in_maps(query, keys, Wq, Wk, Wv, gamma, beta)
    res = _run(in_maps)
    output = np.empty((B, S, D), dtype=np.float32)
    for c in range(8):
        b, g = c % B, c // B
        oc = res.results[c]["out"]                   # (128, 4, 2048)
        block = oc.transpose(2, 1, 0).reshape(S, GF)  # (S, GF): [t, h*128+p]
        output[b, :, GF * g : GF * (g + 1)] = block
    return output


# revision 16
# speedup vs baseline: 1.4125x; 1.0552x over previous
"""Multi-head attention + residual + batchnorm on 8 trn2 NeuronCores.

Sharding: core c handles batch b = c % 4 and head-group g = c // 4
(4 heads = 512 feature dims per group). All device compute happens in
feature-major ("transposed") space so every matmul contracts over the
partition dim with zero on-chip transposes:

  QT[u,t] = (Wq_g/sqrt(D)) @ query[b].T      (fp32r)
  KT[u,t] = Wk_g @ keys[b].T                 (fp32r)
  V[t,u]  = keys[b] @ Wv_g.T                 (fp32r in, f32 psum, bf16 out)
  ST[k,q] = KT_h.T-contract QT_h             (fp32r; scores transposed)
  PT      = exp(ST)            (ACT, PSUM->SBUF, bf16; scores bounded, no max)
  OT[u,q] = sum_k V[k,u]*PT[k,q]             (bf16)
  r[q]    = sum_k PT[k,q]  via ones-matmul   (bf16)
  o_res   = OT/r + query[b].T slice          (f32)
  batchnorm over (b,s): local sums + 4-core AllReduce, then affine.

BatchNorm feature stats are local to a head-group, reduced across the 4
cores sharing g (replica groups [[0..3],[4..7]]).
"""
import sys

sys.path.insert(0, "/opt/trn_rl_repo")

import numpy as np

import concourse.bass as bass
import concourse.tile as tile
from concourse import bacc, mybir
from concourse.bass_utils import run_bass_kernel_spmd

F32 = mybir.dt.float32
F32R = mybir.dt.float32r
BF16 = mybir.dt.bfloat16
AF = mybir.ActivationFunctionType

B, S, D, H = 4, 2048, 1024, 8
DH = D // H          # 128
HG = 4               # heads per group (per core)
GF = HG * DH         # 512 features per group
EPS = 1e-5
P = 128
DT = 8               # d-tiles (D / 128)
TC = 4               # token chunks of 512
TCW = 512
KT_N = 16            # k tiles of 128 per sequence
NTOK = B * S         # batchnorm population per feature


def _build():
    nc = bacc.Bacc(num_swdge_queues=8)
    qt = nc.declare_dram_parameter("qt", [D, S], F32R, isOutput=False)
    kt = nc.declare_dram_parameter("kt", [D, S], F32R, isOutput=False)
    wq = nc.declare_dram_parameter("wq", [D, GF], F32R, isOutput=False)
    wk = nc.declare_dram_parameter("wk", [D, GF], F32R, isOutput=False)
    wv = nc.declare_dram_parameter("wv", [D, GF], F32R, isOutput=False)
    qres = nc.declare_dram_parameter("qres", [GF, S], F32, isOutput=False)
    gamma = nc.declare_dram_parameter("gamma", [P, HG], F32, isOutput=False)
    beta = nc.declare_dram_parameter("beta", [P, HG], F32, isOutput=False)
    out = nc.declare_dram_parameter("out", [P, HG, S], F32, isOutput=True)

    qt_t = qt.rearrange("(dt p) t -> p dt t", p=P)      # (128, 8, 2048)
    kt_t = kt.rearrange("(dt p) t -> p dt t", p=P)
    qres_t = qres.rearrange("(h p) t -> p h t", p=P)    # (128, 4, 2048)

    with tile.TileContext(nc) as tc:
        with (
            tc.tile_pool(name="persist", bufs=1) as persist,
            tc.tile_pool(name="dram", bufs=1, space="DRAM") as dram,
        ):
            # ---- persistent SBUF ----
            QT = persist.tile([P, HG, S], BF16)          # (dh, h, q) 16KB/p
            KTb = persist.tile([P, HG, S], BF16)         # (dh, h, k) 16KB/p
            V = persist.tile([P, KT_N, GF], BF16)        # (t128, kt, u) 16KB/p
            o_res = persist.tile([P, HG, S], F32)        # 32KB/p
            gam = persist.tile([P, HG], F32)
            bet = persist.tile([P, HG], F32)
            ones_f = persist.tile([P, P], F32)
            ones_b = persist.tile([P, P], F32R)
            cc_in = [dram.tile([P, 2], F32, name=f"cc_in{h}") for h in range(HG)]
            cc_out = [dram.tile([P, 2], F32, name=f"cc_out{h}") for h in range(HG)]

            nc.sync.dma_start(gam[:], gamma[:])
            nc.sync.dma_start(bet[:], beta[:])
            eps_t = persist.tile([P, 1], F32)
            nc.vector.memset(eps_t[:], float(EPS))
            nc.vector.memset(ones_f[:], 1.0)
            nc.vector.tensor_copy(ones_b[:], ones_f[:])

            # ---- phase 1: projections (stream qt/kt, weights resident) ----
            with (
                tc.tile_pool(name="wpool", bufs=1) as wpool,
                tc.tile_pool(name="xstream", bufs=20) as xstream,
                tc.tile_pool(name="ppsum", bufs=4, space="PSUM") as ppsum,
                tc.tile_pool(name="ptmp", bufs=4) as ptmp,
            ):
                wq_s = [wpool.tile([P, GF], F32R, name=f"wq{d}") for d in range(DT)]
                wk_s = [wpool.tile([P, GF], F32R, name=f"wk{d}") for d in range(DT)]
                wv_s = [wpool.tile([P, GF], F32R, name=f"wv{d}") for d in range(DT)]
                wq_r = wq.rearrange("(dt p) u -> dt p u", p=P)
                wk_r = wk.rearrange("(dt p) u -> dt p u", p=P)
                wv_r = wv.rearrange("(dt p) u -> dt p u", p=P)
                for d in range(DT):
                    nc.sync.dma_start(wq_s[d][:], wq_r[d])

                # Q^T
                for tc_i in range(TC):
                    xh = []
                    if tc_i == 1:
                        for d in range(DT):
                            nc.sync.dma_start(wk_s[d][:], wk_r[d])
                    elif tc_i == 2:
                        for d in range(DT):
                            nc.sync.dma_start(wv_s[d][:], wv_r[d])
                    for d in range(DT):
                        t = xstream.tile([P, TCW], F32R, tag="x")
                        nc.sync.dma_start(t[:], qt_t[:, d, bass.ts(tc_i, TCW)])
                        xh.append(t)
                    for h in range(HG):
                        ps = ppsum.tile([P, TCW], F32)
                        for d in range(DT):
                            nc.tensor.matmul(
                                ps[:],
                                wq_s[d][:, bass.ts(h, DH)],
                                xh[d][:],
                                start=(d == 0),
                                stop=(d == DT - 1),
                            )
                        nc.scalar.copy(
                            QT[:, h, bass.ts(tc_i, TCW)], ps[:]
                        )
                # K^T and V
                for tc_i in range(TC):
                    xh = []
                    for d in range(DT):
                        t = xstream.tile([P, TCW], F32R, tag="x")
                        nc.sync.dma_start(t[:], kt_t[:, d, bass.ts(tc_i, TCW)])
                        xh.append(t)
                    for h in range(HG):
                        ps = ppsum.tile([P, TCW], F32)
                        for d in range(DT):
                            nc.tensor.matmul(
                                ps[:],
                                wk_s[d][:, bass.ts(h, DH)],
                                xh[d][:],
                                start=(d == 0),
                                stop=(d == DT - 1),
                            )
                        nc.scalar.copy(
                            KTb[:, h, bass.ts(tc_i, TCW)], ps[:]
                        )
                    for sub in range(TCW // P):  # 4 t128 tiles in this chunk
                        kt_idx = tc_i * (TCW // P) + sub
                        psv = ppsum.tile([P, GF], F32)
                        for d in range(DT):
                            nc.tensor.matmul(
                                psv[:],
                                xh[d][:, bass.ts(sub, P)],
                                wv_s[d][:],
                                start=(d == 0),
                                stop=(d == DT - 1),
                            )
                        nc.scalar.copy(V[:, kt_idx, :], psv[:])

            # ---- phase 2: attention + residual ----
            with (
                tc.tile_pool(name="pt_pool", bufs=2) as pt_pool,
                tc.tile_pool(name="qr_pool", bufs=3) as qr_pool,
                tc.tile_pool(name="spsum", bufs=2, space="PSUM") as spsum,
                tc.tile_pool(name="opsum", bufs=2, space="PSUM") as opsum,
                tc.tile_pool(name="rpsum", bufs=2, space="PSUM") as rpsum,
                tc.tile_pool(name="small", bufs=4) as small,
            ):
                for h in range(HG):
                    bstat = small.tile(
                        [P, TC, nc.vector.BN_STATS_DIM], F32, tag="bstat"
                    )
                    for q_i in range(TC):
                        PT = pt_pool.tile([P, KT_N, TCW], BF16, tag="pt")
                        for kp in range(KT_N // 2):
                            ps_s = spsum.tile([P, 2, TCW], F32, tag="s")
                            for j in range(2):
                                nc.tensor.matmul(
                                    ps_s[:, j, :],
                                    KTb[:, h, bass.ts(2 * kp + j, P)],
                                    QT[:, h, bass.ts(q_i, TCW)],
                                    start=True,
                                    stop=True,
                                )
                            nc.scalar.activation(
                                out=PT[:, 2 * kp : 2 * kp + 2, :],
                                in_=ps_s[:],
                                func=AF.Exp,
                            )
                        ps_o = opsum.tile([P, TCW], F32, tag="o")
                        for k in range(KT_N):
                            nc.tensor.matmul(
                                ps_o[:],
                                V[:, k, bass.ts(h, DH)],
                                PT[:, k, :],
                                start=(k == 0),
                                stop=(k == KT_N - 1),
                            )
                        # pairwise in-place tree over the 16 k-tiles of PT
                        for step in (1, 2, 4, 8):
                            for j in range(0, KT_N, 2 * step):
                                if step < 8:
                                    nc.vector.tensor_add(
                                        PT[:, j, :], PT[:, j, :], PT[:, j + step, :]
                                    )
                        rpart = small.tile([P, TCW], F32R, tag="rpart")
                        nc.vector.tensor_add(
                            rpart[:], PT[:, 0, :], PT[:, 8, :]
                        )
                        ps_r = rpsum.tile([P, TCW], F32, tag="r")
                        nc.tensor.matmul(
                            ps_r[:], ones_b[:], rpart[:], start=True, stop=True
                        )
                        rb = small.tile([P, TCW], F32, tag="rb")
                        nc.vector.reciprocal_approx_fast(out=rb[:], in_=ps_r[:])
                        qres_ch = qr_pool.tile([P, TCW], F32, tag="qres")
                        nc.sync.dma_start(
                            qres_ch[:], qres_t[:, h, bass.ts(q_i, TCW)]
                        )
                        dst = o_res[:, h, bass.ts(q_i, TCW)]
                        nc.vector.tensor_tensor(
                            dst, ps_o[:], rb[:], mybir.AluOpType.mult
                        )
                        nc.vector.tensor_add(dst, dst, qres_ch[:])
                        nc.vector.bn_stats(out=bstat[:, q_i, :], in_=dst)

                    # per-head all-reduce + affine + writeout (overlaps later heads)
                    mv = small.tile([P, 2], F32, tag="mv")
                    nc.vector.bn_aggr(out=mv[:], in_=bstat[:])
                    # sh = [sum, sumsq] = [mean*S, (var + mean^2)*S]
                    sh = small.tile([P, 2], F32, tag="sh")
                    nc.vector.tensor_scalar_mul(sh[:, 0:1], mv[:, 0:1], float(S))
                    sq = small.tile([P, 1], F32, tag="sq")
                    nc.vector.tensor_mul(sq[:], mv[:, 0:1], mv[:, 0:1])
                    nc.vector.tensor_add(sq[:], sq[:], mv[:, 1:2])
                    nc.vector.tensor_scalar_mul(sh[:, 1:2], sq[:], float(S))

                    nc.gpsimd.dma_start(cc_in[h][:], sh[:])
                    nc.gpsimd.collective_compute(
                        "AllReduce",
                        mybir.AluOpType.add,
                        ins=[cc_in[h].opt()],
                        outs=[cc_out[h].opt()],
                        replica_groups=[[0, 1, 2, 3], [4, 5, 6, 7]],
                    )
                    gstat = small.tile([P, 2], F32, tag="gstat")
                    nc.gpsimd.dma_start(gstat[:], cc_out[h][:])

                    # mean = sum/NTOK ; var = sumsq/NTOK - mean^2
                    mean = small.tile([P, 1], F32, tag="mean")
                    var = small.tile([P, 1], F32, tag="var")
                    nc.vector.tensor_scalar_mul(mean[:], gstat[:, 0:1], 1.0 / NTOK)
                    nc.vector.tensor_scalar_mul(var[:], gstat[:, 1:2], 1.0 / NTOK)
                    msq = small.tile([P, 1], F32, tag="msq")
                    nc.vector.tensor_mul(msq[:], mean[:], mean[:])
                    nc.vector.tensor_sub(var[:], var[:], msq[:])
                    std = small.tile([P, 1], F32, tag="std")
                    nc.scalar.activation(
                        out=std[:], in_=var[:], func=AF.Sqrt, bias=eps_t[:]
                    )
                    rstd = small.tile([P, 1], F32, tag="rstd")
                    nc.vector.reciprocal(out=rstd[:], in_=std[:])
                    scale = small.tile([P, 1], F32, tag="scale")
                    shift = small.tile([P, 1], F32, tag="shift")
                    nc.vector.tensor_mul(scale[:], gam[:, h : h + 1], rstd[:])
                    nc.vector.tensor_mul(shift[:], mean[:], scale[:])
                    nc.vector.tensor_sub(shift[:], bet[:, h : h + 1], shift[:])

                    nc.vector.tensor_scalar(
                        o_res[:, h, :],
                        o_res[:, h, :],
                        scale[:],
                        shift[:],
                        mybir.AluOpType.mult,
                        mybir.AluOpType.add,
                    )
                    nc.sync.dma_start(out[:, h, :], o_res[:, h, :])

    nc.finalize()
    return nc


_NC = None


def _get_nc():
    global _NC
    if _NC is None:
        _NC = _build()
    return _NC


def _make_in_maps(query, keys, Wq, Wk, Wv, gamma, beta):
    query = np.asarray(query, dtype=np.float32)
    keys = np.asarray(keys, dtype=np.float32)
    Wq = np.asarray(Wq, dtype=np.float32)
    Wk = np.asarray(Wk, dtype=np.float32)
    Wv = np.asarray(Wv, dtype=np.float32)
    gamma = np.asarray(gamma, dtype=np.float32)
    beta = np.asarray(beta, dtype=np.float32)

    scale = 1.0 / np.sqrt(np.float32(D))
    in_maps = []
    for c in range(8):
        b, g = c % B, c // B
        rows = slice(GF * g, GF * (g + 1))
        qt = np.ascontiguousarray(query[b].T)              # (D, S)
        kt = np.ascontiguousarray(keys[b].T)
        in_maps.append(
            {
                "qt": qt,
                "kt": kt,
                "wq": np.ascontiguousarray(Wq[rows].T * scale),  # (D, GF)
                "wk": np.ascontiguousarray(Wk[rows].T),
                "wv": np.ascontiguousarray(Wv[rows].T),
                "qres": np.ascontiguousarray(qt[rows]),          # (GF, S)
                "gamma": np.ascontiguousarray(
                    gamma[rows].reshape(HG, P).T
                ),
                "beta": np.ascontiguousarray(beta[rows].reshape(HG, P).T),
            }
        )
    return in_maps


def _run(in_maps, trace=False, **kw):
    nc = _get_nc()
    return run_bass_kernel_spmd(
        nc, in_maps, core_ids=list(range(8)), trace=trace, **kw
    )


def kernel(query, keys, Wq, Wk, Wv, gamma, beta):
    in_maps = _make_in_maps(query, keys, Wq, Wk, Wv, gamma, beta)
    res = _run(in_maps)
    output = np.empty((B, S, D), dtype=np.float32)
    for c in range(8):
        b, g = c % B, c // B
        oc = res.results[c]["out"]                   # (128, 4, 2048)
        block = oc.transpose(2, 1, 0).reshape(S, GF)  # (S, GF): [t, h*128+p]
        output[b, :, GF * g : GF * (g + 1)] = block
    return output
